# revision 20
# baseline (speedup 1.0000x reference)
"""Block sliding-window attention on 8 TRN2 NeuronCores.

Sharding: sequence-parallel. 8 shards = (batch b in {0,1}) x (quarter s in
0..3); each core owns 2048 consecutive tokens of one batch plus a 256-token
K/V halo from the previous quarter (zeros + -inf gate for the first quarter).
No collectives: each core computes its tokens' full output rows.

Engine-cost notes driving the layout: every bf16-stationary matmul emits a
separate Ldweights (~146ns serial PE.SEQ); f32r stationaries self-load.
Walrus forbids mixing 16/32-bit matmul operands, and PSUM matmul outputs are
capped at one bank (512 f32). DMA instructions serialize ~625ns on HWDGE, so
transfers are batched into few big copies.

  P1 (all f32r, zero Ldweights): V = hidden @ Wv (256-col Wv panels),
      KT/QT = W^T @ hiddenT per head, staged to DRAM scratch bf16.
  P2+P3 fused per 256-token chunk: RoPE on Q/K (DVE, bf16), per head:
      S^T = K Q^T (bf16), exp on ACT -> f32r probs (pgate bias gates chunk
      0's no-previous case), triangular mask mul in-place on DVE,
      denominator pre-add on Pool + ones-matmul (f32r), O^T = V^T P^T
      (f32r; V cast bf16->f32r via gpsimd casting DMA), normalize via DVE
      reciprocal+mul, then out[chunk] = sum_h O_h @ Wo_h (bf16, resident
      Wo) accumulated in PSUM and stored straight to OUT.
"""
import sys

try:
    import concourse  # noqa: F401
except ImportError:
    sys.path.insert(0, '/opt/trn_rl_repo')

import ml_dtypes
import numpy as np

import concourse.bacc as bacc
import concourse.mybir as mybir
import concourse.tile as tile
from concourse.bass_utils import run_bass_kernel_spmd

f32 = mybir.dt.float32
f32r = mybir.dt.float32r
bf16 = mybir.dt.bfloat16
AF = mybir.ActivationFunctionType

DIMS = 2048
HEADS = 16
HD = 128           # head dim
WIN = 256          # window / chunk
B, S = 2, 8192
NSH = 4            # seq shards per batch
THETA = 10000.0
ISQ = float(1.0 / np.sqrt(HD))
IB = DIMS // 128   # 16 input-dim blocks


def tok_tiles(n):
    out, a = [], 0
    while a < n:
        w = min(512, n - a)
        out.append((a, w))
        a += w
    return out


def build(nc, T):
    """Emit the per-core program. T = local tokens (multiple of 512)."""
    TH = T + WIN                      # with halo
    NC_ = T // WIN                    # chunks
    NTB = TH // 128                   # 128-token blocks incl halo
    HT = nc.dram_tensor("HT", [DIMS, TH], f32r, kind="ExternalInput")
    WQ = nc.dram_tensor("WQ", [DIMS, DIMS], f32r, kind="ExternalInput")
    WK = nc.dram_tensor("WK", [DIMS, DIMS], f32r, kind="ExternalInput")
    WV = nc.dram_tensor("WV", [DIMS, DIMS], f32r, kind="ExternalInput")
    WO = nc.dram_tensor("WO", [DIMS, DIMS], bf16, kind="ExternalInput")
    COS4 = nc.dram_tensor("COS4", [128, 4, TH], bf16, kind="ExternalInput")
    SIN4 = nc.dram_tensor("SIN4", [128, 4, TH], bf16, kind="ExternalInput")
    TRI23 = nc.dram_tensor("TRI23", [128, 2 * WIN], bf16, kind="ExternalInput")
    PGATE = nc.dram_tensor("PGATE", [128, 1], f32, kind="ExternalInput")
    ONESM = nc.dram_tensor("ONESM", [128, 128], f32r, kind="ExternalInput")
    OUT = nc.dram_tensor("OUT", [T, DIMS], f32, kind="ExternalOutput")

    QTS = nc.dram_tensor("QTS", [HEADS, HD, T], bf16)    # raw (pre-RoPE) Q^T
    KTS = nc.dram_tensor("KTS", [HEADS, HD, TH], bf16)   # raw K^T (with halo)
    VS = nc.dram_tensor("VS", [TH, DIMS], bf16)          # V natural

    with tile.TileContext(nc) as tc:
        with tc.tile_pool(name="cst", bufs=1) as cst:
            tri23 = cst.tile([128, 2 * WIN], bf16)
            pgate = cst.tile([128, 1], f32)
            onesm = cst.tile([128, 128], f32r)
            nc.sync.dma_start(tri23[:], TRI23[:])
            nc.sync.dma_start(pgate[:], PGATE[:])
            nc.sync.dma_start(onesm[:], ONESM[:])

            # ---------------- P1: projections (all f32r) ----------------
            with tc.tile_pool(name="p1", bufs=1) as p1:
                ht = p1.tile([128, IB, TH], f32r)
                htr = HT.rearrange("(ib p) t -> p ib t", p=128)
                tg = TH // 8
                for g in range(8):
                    nc.sync.dma_start(ht[:, :, g * tg:(g + 1) * tg],
                                      htr[:, :, g * tg:(g + 1) * tg])

                # V natural: lhsT = hT block [128in, 128tok], rhs = Wv panel
                with tc.tile_pool(name="wv", bufs=2) as wvp, \
                     tc.tile_pool(name="vb", bufs=2) as vbp, \
                     tc.tile_pool(name="vps", bufs=8, space="PSUM") as vps:
                    wvr = WV.rearrange("(ib p) o -> p ib o", p=128)
                    vsr = VS.rearrange("(tb p) c -> p tb c", p=128)
                    for og in range(8):
                        wv = wvp.tile([128, IB, 256], f32r, name="wv")
                        nc.sync.dma_start(
                            wv[:], wvr[:, :, og * 256:(og + 1) * 256])
                        vbog = vbp.tile([128, NTB, 256], bf16, name="vb")
                        for tb in range(NTB):
                            ps = vps.tile([128, 256], f32, name="vps")
                            for ib in range(IB):
                                nc.tensor.matmul(
                                    ps[:], ht[:, ib, tb * 128:(tb + 1) * 128],
                                    wv[:, ib, :],
                                    start=(ib == 0), stop=(ib == IB - 1))
                            nc.vector.tensor_copy(vbog[:, tb, :], ps[:])
                        nc.sync.dma_start(
                            vsr[:, :, og * 256:(og + 1) * 256], vbog[:])

                # KT then QT: lhsT = W block [128in, 128out], rhs = hT
                with tc.tile_pool(name="wkq", bufs=3) as wkqp, \
                     tc.tile_pool(name="kst", bufs=3) as kstp, \
                     tc.tile_pool(name="kqps", bufs=8, space="PSUM") as kqps:
                    for W_, DST, t0, tlen in ((WK, KTS, 0, TH),
                                              (WQ, QTS, WIN, T)):
                        wr = W_.rearrange("(ib p) o -> p ib o", p=128)
                        tts = tok_tiles(tlen)
                        for ob in range(HEADS):
                            wt = wkqp.tile([128, IB, 128], f32r, name="wkq")
                            nc.sync.dma_start(
                                wt[:], wr[:, :, ob * 128:(ob + 1) * 128])
                            st = kstp.tile([128, TH], bf16, name="kst")
                            psums = [kqps.tile([128, 512], f32, name="kqps")
                                     for _ in tts]
                            for ib in range(IB):
                                for ti, (a, w) in enumerate(tts):
                                    nc.tensor.matmul(
                                        psums[ti][:, :w], wt[:, ib, :],
                                        ht[:, ib, t0 + a:t0 + a + w],
                                        start=(ib == 0), stop=(ib == IB - 1))
                            for ti, (a, w) in enumerate(tts):
                                nc.scalar.copy(st[:, a:a + w], psums[ti][:, :w])
                            nc.sync.dma_start(DST[ob][:, :], st[:, :tlen])

            # ---------------- P2 + P3 fused ----------------
            with tc.tile_pool(name="wop", bufs=1) as wop, \
                 tc.tile_pool(name="qk", bufs=2) as qk, \
                 tc.tile_pool(name="rt", bufs=2) as rt, \
                 tc.tile_pool(name="tp", bufs=1) as tp, \
                 tc.tile_pool(name="ptp", bufs=2) as ptp, \
                 tc.tile_pool(name="pa", bufs=2) as pap, \
                 tc.tile_pool(name="ob", bufs=2) as obp, \
                 tc.tile_pool(name="ot", bufs=9) as otp, \
                 tc.tile_pool(name="so3", bufs=2) as so3p, \
                 tc.tile_pool(name="ps_s", bufs=2, space="PSUM") as ps_s, \
                 tc.tile_pool(name="ps_d", bufs=2, space="PSUM") as ps_d, \
                 tc.tile_pool(name="ps_o", bufs=2, space="PSUM") as ps_o, \
                 tc.tile_pool(name="pp3", bufs=2, space="PSUM") as pp3:
                wo = wop.tile([128, IB, DIMS], bf16)
                wor = WO.rearrange("(ib p) o -> p ib o", p=128)
                for g in range(4):
                    nc.sync.dma_start(wo[:, g * 4:(g + 1) * 4, :],
                                      wor[:, g * 4:(g + 1) * 4, :])

                def rope_load(SRC, c0, roped, pos0=None):
                    """Load [128, HEADS, WIN] token window at c0 from SRC
                    (head-major scratch), apply RoPE into `roped` (bf16).
                    pos0: column into COS4/SIN4 (halo coords); default c0."""
                    if pos0 is None:
                        pos0 = c0
                    raw = rt.tile([128, HEADS, WIN], bf16, name="raw")
                    rot = rt.tile([128, HEADS, WIN], bf16, name="rot")
                    sl = SRC[:, :, c0:c0 + WIN]
                    nc.sync.dma_start(raw[:], sl.rearrange("h d w -> d h w"))
                    nc.sync.dma_start(
                        rot[0:64], sl[:, 64:128, :].rearrange("h d w -> d h w"))
                    nc.sync.dma_start(
                        rot[64:128], sl[:, 0:64, :].rearrange("h d w -> d h w"))
                    cos4 = tp.tile([128, 4, WIN], bf16, name="cos4")
                    sin4 = tp.tile([128, 4, WIN], bf16, name="sin4")
                    nc.sync.dma_start(cos4[:], COS4[:, :, pos0:pos0 + WIN])
                    nc.sync.dma_start(sin4[:], SIN4[:, :, pos0:pos0 + WIN])
                    for g in range(4):
                        gs = slice(g * 4, (g + 1) * 4)
                        tmp = tp.tile([128, 4, WIN], bf16, name="tmp")
                        nc.vector.tensor_mul(tmp[:], rot[:, gs], sin4[:])
                        nc.vector.tensor_mul(roped[:, gs], raw[:, gs], cos4[:])
                        nc.vector.tensor_add(roped[:, gs], roped[:, gs], tmp[:])

                kt_prev = qk.tile([128, HEADS, WIN], bf16, name="kt")
                rope_load(KTS, 0, kt_prev)
                v_prev = qk.tile([128, 2, DIMS], f32r, name="v")
                nc.gpsimd.dma_start(
                    v_prev[:], VS[0:WIN].rearrange("(tb p) c -> p tb c", p=128))

                W2 = 2 * WIN
                for c in range(NC_):
                    kt_cur = qk.tile([128, HEADS, WIN], bf16, name="kt")
                    rope_load(KTS, WIN + c * WIN, kt_cur)
                    v_cur = qk.tile([128, 2, DIMS], f32r, name="v")
                    nc.gpsimd.dma_start(
                        v_cur[:], VS[WIN + c * WIN:WIN + (c + 1) * WIN]
                        .rearrange("(tb p) c -> p tb c", p=128))
                    qt = qk.tile([128, HEADS, WIN], bf16, name="qt")
                    rope_load(QTS, c * WIN, qt, pos0=WIN + c * WIN)

                    kts = [kt_prev, kt_prev, kt_cur, kt_cur]
                    vs = [v_prev, v_prev, v_cur, v_cur]
                    ots = []
                    for h0 in range(0, HEADS, 2):
                        pd = ps_d.tile([128, W2], f32, name="pd")
                        po = ps_o.tile([128, W2], f32, name="po")
                        pbs2, pads = [], []
                        for h in (h0, h0 + 1):
                            pbs = []
                            for pr in range(2):
                                ps = ps_s.tile([128, W2], f32, name="ps")
                                for kb2 in range(2):
                                    kb = pr * 2 + kb2
                                    nc.tensor.matmul(
                                        ps[:, kb2 * WIN:(kb2 + 1) * WIN],
                                        kts[kb][:, h,
                                                (kb % 2) * 128:(kb % 2) * 128 + 128],
                                        qt[:, h], start=True, stop=True)
                                pb = ptp.tile([128, W2], f32r, name=f"pt{pr}")
                                if pr == 0:
                                    if c == 0:
                                        nc.scalar.activation(
                                            pb[:], ps[:], AF.Exp,
                                            bias=pgate[:], scale=ISQ)
                                    else:
                                        nc.scalar.activation(
                                            pb[:], ps[:], AF.Exp, scale=ISQ)
                                else:
                                    nc.scalar.activation(pb[:], ps[:], AF.Exp,
                                                         scale=ISQ)
                                    nc.vector.tensor_mul(pb[:], pb[:],
                                                         tri23[:])
                                pbs.append(pb)
                            pbs2.append(pbs)
                            pad = pap.tile([128, W2], f32r, name="pad")
                            nc.gpsimd.tensor_add(pad[:], pbs[0][:], pbs[1][:])
                            pads.append(pad)

                        for i, h in enumerate((h0, h0 + 1)):
                            sl = slice(i * WIN, (i + 1) * WIN)
                            for half in range(2):
                                nc.tensor.matmul(
                                    pd[:, sl], onesm[:],
                                    pads[i][:, half * WIN:(half + 1) * WIN],
                                    start=(half == 0), stop=(half == 1))
                            for kb in range(4):
                                pb = pbs2[i][kb // 2][
                                    :, (kb % 2) * WIN:(kb % 2 + 1) * WIN]
                                nc.tensor.matmul(
                                    po[:, sl],
                                    vs[kb][:, kb % 2, h * 128:(h + 1) * 128],
                                    pb, start=(kb == 0), stop=(kb == 3))
                        rb = obp.tile([128, W2], f32, name="rb")
                        with nc.allow_low_precision("softmax denominator"):
                            nc.vector.reciprocal(rb[:], pd[:])
                        ot = otp.tile([128, W2], bf16, name="ot")
                        nc.vector.tensor_mul(ot[:], po[:], rb[:])
                        ots.append(ot)

                    # P3 for this chunk's 256 tokens
                    for tt in range(2):
                        for nt in range(4):
                            ps3 = pp3.tile([128, 512], f32, name="pp3")
                            for h in range(HEADS):
                                hp, i = divmod(h, 2)
                                lhs = ots[hp][:, i * WIN + tt * 128:
                                              i * WIN + tt * 128 + 128]
                                nc.tensor.matmul(
                                    ps3[:], lhs,
                                    wo[:, h, nt * 512:(nt + 1) * 512],
                                    start=(h == 0), stop=(h == HEADS - 1))
                            so = so3p.tile([128, 512], f32, name="so")
                            nc.scalar.copy(so[:], ps3[:])
                            nc.sync.dma_start(
                                OUT[c * WIN + tt * 128:
                                    c * WIN + (tt + 1) * 128,
                                    nt * 512:(nt + 1) * 512], so[:])
                    kt_prev, v_prev = kt_cur, v_cur
    return nc


def _host_inputs(hidden_states, Wq, Wk, Wv, Wo, T):
    """Build the 8 per-core input maps."""
    TH = T + WIN
    inv_freq = 1.0 / (THETA ** (np.arange(0, HD, 2, dtype=np.float32) / HD))

    qq = np.arange(WIN)[None, :]
    kk = np.arange(128)[:, None]
    tri23 = np.concatenate([(qq >= kk), (qq >= kk + 128)], 1).astype(
        ml_dtypes.bfloat16)
    onesm_f32 = np.ones((128, 128), np.float32)

    wq32, wk32, wv32 = (np.ascontiguousarray(w, np.float32)
                        for w in (Wq, Wk, Wv))
    wo_bf = np.ascontiguousarray(np.asarray(Wo).astype(ml_dtypes.bfloat16))
    in_maps = []
    for core in range(8):
        b, sh = divmod(core, NSH)
        t0 = sh * T
        hs = np.zeros((TH, DIMS), np.float32)
        lo = max(0, t0 - WIN)
        hs[WIN - (t0 - lo):] = hidden_states[b, lo:t0 + T]
        hT = np.ascontiguousarray(hs.T)

        pos = np.arange(t0 - WIN, t0 + T, dtype=np.float32)
        f = np.outer(inv_freq, pos)                      # [64, TH]
        cos = np.concatenate([np.cos(f), np.cos(f)], 0)  # [128, TH]
        sin = np.sin(f)
        sins = np.concatenate([-sin, sin], 0)
        cos4 = np.ascontiguousarray(
            np.broadcast_to(cos[:, None, :], (128, 4, TH)).astype(
                ml_dtypes.bfloat16))
        sin4 = np.ascontiguousarray(
            np.broadcast_to(sins[:, None, :], (128, 4, TH)).astype(
                ml_dtypes.bfloat16))
        pg = np.full((128, 1), -1e30 if sh == 0 else 0.0, np.float32)
        in_maps.append({
            "HT": hT, "WQ": wq32, "WK": wk32, "WV": wv32, "WO": wo_bf,
            "COS4": cos4, "SIN4": sin4,
            "TRI23": tri23, "PGATE": pg, "ONESM": onesm_f32,
        })
    return in_maps


_CACHE = {}


def run(hidden_states, Wq, Wk, Wv, Wo, T=S // NSH, **spmd_kwargs):
    key = T
    if key not in _CACHE:
        nc = bacc.Bacc(None)
        build(nc, T)
        nc.finalize()
        _CACHE[key] = nc
    nc = _CACHE[key]
    in_maps = _host_inputs(hidden_states, Wq, Wk, Wv, Wo, T)
    res = run_bass_kernel_spmd(nc, in_maps, core_ids=list(range(8)),
                               **spmd_kwargs)
    outs = [res.results[i]["OUT"] for i in range(8)]
    full = np.empty((B, NSH * T, DIMS), np.float32)
    for core in range(8):
        b, sh = divmod(core, NSH)
        full[b, sh * T:(sh + 1) * T] = outs[core]
    return full, res


def kernel(hidden_states, Wq, Wk, Wv, Wo):
    out, _ = run(np.asarray(hidden_states), Wq, Wk, Wv, Wo)
    return out


# revision 24
# speedup vs baseline: 1.1051x; 1.1051x over previous
"""Block sliding-window attention on 8 TRN2 NeuronCores.

Sharding: sequence-parallel. 8 shards = (batch b in {0,1}) x (quarter s in
0..3); each core owns 2048 consecutive tokens of one batch plus a 256-token
K/V halo from the previous quarter (zeros + -inf gate for the first quarter).
No collectives: each core computes its tokens' full output rows.

Engine-cost notes driving the layout: every bf16-stationary matmul emits a
separate Ldweights (~146ns serial PE.SEQ); f32r stationaries self-load.
Walrus forbids mixing 16/32-bit matmul operands, and PSUM matmul outputs are
capped at one bank (512 f32). DMA instructions serialize ~625ns on HWDGE, so
transfers are batched into few big copies.

  P1 (all f32r, zero Ldweights): V = hidden @ Wv (256-col Wv panels),
      KT/QT = W^T @ hiddenT per head, staged to DRAM scratch bf16.
  P2+P3 fused per 256-token chunk: RoPE on Q/K (DVE, bf16), per head:
      S^T = K Q^T (bf16), exp on ACT -> f32r probs (pgate bias gates chunk
      0's no-previous case), triangular mask mul in-place on DVE,
      denominator pre-add on Pool + ones-matmul (f32r), O^T = V^T P^T
      (f32r; V cast bf16->f32r via gpsimd casting DMA), normalize via DVE
      reciprocal+mul, then out[chunk] = sum_h O_h @ Wo_h (bf16, resident
      Wo) accumulated in PSUM and stored straight to OUT.
"""
import sys

try:
    import concourse  # noqa: F401
except ImportError:
    sys.path.insert(0, '/opt/trn_rl_repo')

import ml_dtypes
import numpy as np

import concourse.bacc as bacc
import concourse.mybir as mybir
import concourse.tile as tile
from concourse.bass_utils import run_bass_kernel_spmd

f32 = mybir.dt.float32
f32r = mybir.dt.float32r
bf16 = mybir.dt.bfloat16
AF = mybir.ActivationFunctionType

DIMS = 2048
HEADS = 16
HD = 128           # head dim
WIN = 256          # window / chunk
B, S = 2, 8192
NSH = 4            # seq shards per batch
THETA = 10000.0
ISQ = float(1.0 / np.sqrt(HD))
IB = DIMS // 128   # 16 input-dim blocks


def tok_tiles(n):
    out, a = [], 0
    while a < n:
        w = min(512, n - a)
        out.append((a, w))
        a += w
    return out


def build(nc, T):
    """Emit the per-core program. T = local tokens (multiple of 512)."""
    TH = T + WIN                      # with halo
    NC_ = T // WIN                    # chunks
    NTB = TH // 128                   # 128-token blocks incl halo
    HT = nc.dram_tensor("HT", [DIMS, TH], f32r, kind="ExternalInput")
    WQ = nc.dram_tensor("WQ", [DIMS, DIMS], f32r, kind="ExternalInput")
    WK = nc.dram_tensor("WK", [DIMS, DIMS], f32r, kind="ExternalInput")
    WV = nc.dram_tensor("WV", [DIMS, DIMS], f32r, kind="ExternalInput")
    WO = nc.dram_tensor("WO", [DIMS, DIMS], bf16, kind="ExternalInput")
    COS4 = nc.dram_tensor("COS4", [128, 4, TH], bf16, kind="ExternalInput")
    SIN4 = nc.dram_tensor("SIN4", [128, 4, TH], bf16, kind="ExternalInput")
    TRI23 = nc.dram_tensor("TRI23", [128, 2 * WIN], bf16, kind="ExternalInput")
    PGATE = nc.dram_tensor("PGATE", [128, 1], f32, kind="ExternalInput")
    ONESM = nc.dram_tensor("ONESM", [128, 128], f32r, kind="ExternalInput")
    OUT = nc.dram_tensor("OUT", [T, DIMS], f32, kind="ExternalOutput")

    QTS = nc.dram_tensor("QTS", [HEADS, HD, T], bf16)    # raw (pre-RoPE) Q^T
    KTS = nc.dram_tensor("KTS", [HEADS, HD, TH], bf16)   # raw K^T (with halo)
    VS = nc.dram_tensor("VS", [TH, DIMS], bf16)          # V natural

    with tile.TileContext(nc) as tc:
        with tc.tile_pool(name="cst", bufs=1) as cst:
            tri23 = cst.tile([128, 2 * WIN], bf16)
            pgate = cst.tile([128, 1], f32)
            onesm = cst.tile([128, 128], f32r)
            nc.sync.dma_start(tri23[:], TRI23[:])
            nc.sync.dma_start(pgate[:], PGATE[:])
            nc.sync.dma_start(onesm[:], ONESM[:])

            # ---------------- P1: projections (all f32r) ----------------
            with tc.tile_pool(name="p1", bufs=1) as p1:
                ht = p1.tile([128, IB, TH], f32r)
                htr = HT.rearrange("(ib p) t -> p ib t", p=128)
                NG = 16
                tg = TH // NG
                nc.sync.dma_start(ht[:, :, 0:tg], htr[:, :, 0:tg])

                # V natural: lhsT = hT block [128in, 128tok], rhs = Wv panel
                with tc.tile_pool(name="wv", bufs=2) as wvp, \
                     tc.tile_pool(name="vb", bufs=2) as vbp, \
                     tc.tile_pool(name="vps", bufs=8, space="PSUM") as vps:
                    wvr = WV.rearrange("(ib p) o -> p ib o", p=128)
                    vsr = VS.rearrange("(tb p) c -> p tb c", p=128)
                    for og in range(8):
                        wv = wvp.tile([128, IB, 256], f32r, name="wv")
                        nc.sync.dma_start(
                            wv[:], wvr[:, :, og * 256:(og + 1) * 256])
                        if og == 0:
                            for g in range(1, NG):
                                nc.sync.dma_start(
                                    ht[:, :, g * tg:(g + 1) * tg],
                                    htr[:, :, g * tg:(g + 1) * tg])
                        vbog = vbp.tile([128, NTB, 256], bf16, name="vb")
                        for tb in range(NTB):
                            ps = vps.tile([128, 256], f32, name="vps")
                            for ib in range(IB):
                                nc.tensor.matmul(
                                    ps[:], ht[:, ib, tb * 128:(tb + 1) * 128],
                                    wv[:, ib, :],
                                    start=(ib == 0), stop=(ib == IB - 1))
                            nc.vector.tensor_copy(vbog[:, tb, :], ps[:])
                        nc.sync.dma_start(
                            vsr[:, :, og * 256:(og + 1) * 256], vbog[:])

                # KT then QT: lhsT = W block [128in, 128out], rhs = hT
                with tc.tile_pool(name="wkq", bufs=3) as wkqp, \
                     tc.tile_pool(name="kst", bufs=3) as kstp, \
                     tc.tile_pool(name="kqps", bufs=8, space="PSUM") as kqps:
                    for W_, DST, t0, tlen in ((WK, KTS, 0, TH),
                                              (WQ, QTS, WIN, T)):
                        wr = W_.rearrange("(ib p) o -> p ib o", p=128)
                        tts = tok_tiles(tlen)
                        for ob in range(HEADS):
                            wt = wkqp.tile([128, IB, 128], f32r, name="wkq")
                            nc.sync.dma_start(
                                wt[:], wr[:, :, ob * 128:(ob + 1) * 128])
                            st = kstp.tile([128, TH], bf16, name="kst")
                            psums = [kqps.tile([128, 512], f32, name="kqps")
                                     for _ in tts]
                            for ib in range(IB):
                                for ti, (a, w) in enumerate(tts):
                                    nc.tensor.matmul(
                                        psums[ti][:, :w], wt[:, ib, :],
                                        ht[:, ib, t0 + a:t0 + a + w],
                                        start=(ib == 0), stop=(ib == IB - 1))
                            for ti, (a, w) in enumerate(tts):
                                nc.scalar.copy(st[:, a:a + w], psums[ti][:, :w])
                            nc.sync.dma_start(DST[ob][:, :], st[:, :tlen])

            # ---------------- P2 + P3 fused ----------------
            with tc.tile_pool(name="wop", bufs=1) as wop, \
                 tc.tile_pool(name="qk", bufs=2) as qk, \
                 tc.tile_pool(name="rt", bufs=2) as rt, \
                 tc.tile_pool(name="tp", bufs=1) as tp, \
                 tc.tile_pool(name="ptp", bufs=2) as ptp, \
                 tc.tile_pool(name="pa", bufs=2) as pap, \
                 tc.tile_pool(name="ob", bufs=2) as obp, \
                 tc.tile_pool(name="ot", bufs=9) as otp, \
                 tc.tile_pool(name="so3", bufs=2) as so3p, \
                 tc.tile_pool(name="ps_s", bufs=2, space="PSUM") as ps_s, \
                 tc.tile_pool(name="ps_d", bufs=2, space="PSUM") as ps_d, \
                 tc.tile_pool(name="ps_o", bufs=2, space="PSUM") as ps_o, \
                 tc.tile_pool(name="pp3", bufs=2, space="PSUM") as pp3:
                wo = wop.tile([128, IB, DIMS], bf16)
                wor = WO.rearrange("(ib p) o -> p ib o", p=128)

                def rope_load(SRC, c0, roped, pos0=None):
                    """Load [128, HEADS, WIN] token window at c0 from SRC
                    (head-major scratch), apply RoPE into `roped` (bf16).
                    pos0: column into COS4/SIN4 (halo coords); default c0."""
                    if pos0 is None:
                        pos0 = c0
                    raw = rt.tile([128, HEADS, WIN], bf16, name="raw")
                    rot = rt.tile([128, HEADS, WIN], bf16, name="rot")
                    sl = SRC[:, :, c0:c0 + WIN]
                    nc.sync.dma_start(raw[:], sl.rearrange("h d w -> d h w"))
                    nc.sync.dma_start(
                        rot[0:64], sl[:, 64:128, :].rearrange("h d w -> d h w"))
                    nc.sync.dma_start(
                        rot[64:128], sl[:, 0:64, :].rearrange("h d w -> d h w"))
                    cos4 = tp.tile([128, 4, WIN], bf16, name="cos4")
                    sin4 = tp.tile([128, 4, WIN], bf16, name="sin4")
                    nc.sync.dma_start(cos4[:], COS4[:, :, pos0:pos0 + WIN])
                    nc.sync.dma_start(sin4[:], SIN4[:, :, pos0:pos0 + WIN])
                    for g in range(4):
                        gs = slice(g * 4, (g + 1) * 4)
                        tmp = tp.tile([128, 4, WIN], bf16, name="tmp")
                        nc.vector.tensor_mul(tmp[:], rot[:, gs], sin4[:])
                        nc.vector.tensor_mul(roped[:, gs], raw[:, gs], cos4[:])
                        nc.vector.tensor_add(roped[:, gs], roped[:, gs], tmp[:])

                kt_prev = qk.tile([128, HEADS, WIN], bf16, name="kt")
                rope_load(KTS, 0, kt_prev)
                v_prev = qk.tile([128, 2, DIMS], f32r, name="v")
                nc.gpsimd.dma_start(
                    v_prev[:], VS[0:WIN].rearrange("(tb p) c -> p tb c", p=128))

                W2 = 2 * WIN
                for c in range(NC_):
                    qt = qk.tile([128, HEADS, WIN], bf16, name="qt")
                    rope_load(QTS, c * WIN, qt, pos0=WIN + c * WIN)
                    v_cur = qk.tile([128, 2, DIMS], f32r, name="v")
                    nc.gpsimd.dma_start(
                        v_cur[:], VS[WIN + c * WIN:WIN + (c + 1) * WIN]
                        .rearrange("(tb p) c -> p tb c", p=128))
                    kt_cur = qk.tile([128, HEADS, WIN], bf16, name="kt")
                    rope_load(KTS, WIN + c * WIN, kt_cur)
                    if c == 0:
                        # wo quarters, emitted after chunk-0 prep so its bulk
                        # doesn't delay the seam-critical rope loads
                        for g in range(4):
                            nc.sync.dma_start(
                                wo[:, :, g * 512:(g + 1) * 512],
                                wor[:, :, g * 512:(g + 1) * 512])

                    kts = [kt_prev, kt_prev, kt_cur, kt_cur]
                    vs = [v_prev, v_prev, v_cur, v_cur]
                    ots = []
                    for h0 in range(0, HEADS, 2):
                        pd = ps_d.tile([128, W2], f32, name="pd")
                        po = ps_o.tile([128, W2], f32, name="po")
                        pbs2, pads = [], []
                        for h in (h0, h0 + 1):
                            pbs = []
                            for pr in range(2):
                                ps = ps_s.tile([128, W2], f32, name="ps")
                                for kb2 in range(2):
                                    kb = pr * 2 + kb2
                                    nc.tensor.matmul(
                                        ps[:, kb2 * WIN:(kb2 + 1) * WIN],
                                        kts[kb][:, h,
                                                (kb % 2) * 128:(kb % 2) * 128 + 128],
                                        qt[:, h], start=True, stop=True)
                                pb = ptp.tile([128, W2], f32r, name=f"pt{pr}")
                                if pr == 0:
                                    if c == 0:
                                        nc.scalar.activation(
                                            pb[:], ps[:], AF.Exp,
                                            bias=pgate[:], scale=ISQ)
                                    else:
                                        nc.scalar.activation(
                                            pb[:], ps[:], AF.Exp, scale=ISQ)
                                else:
                                    nc.scalar.activation(pb[:], ps[:], AF.Exp,
                                                         scale=ISQ)
                                    nc.vector.tensor_mul(pb[:], pb[:],
                                                         tri23[:])
                                pbs.append(pb)
                            pbs2.append(pbs)
                            pad = pap.tile([128, W2], f32r, name="pad")
                            nc.gpsimd.tensor_add(pad[:], pbs[0][:], pbs[1][:])
                            pads.append(pad)

                        for i, h in enumerate((h0, h0 + 1)):
                            sl = slice(i * WIN, (i + 1) * WIN)
                            for half in range(2):
                                nc.tensor.matmul(
                                    pd[:, sl], onesm[:],
                                    pads[i][:, half * WIN:(half + 1) * WIN],
                                    start=(half == 0), stop=(half == 1))
                            for kb in range(4):
                                pb = pbs2[i][kb // 2][
                                    :, (kb % 2) * WIN:(kb % 2 + 1) * WIN]
                                nc.tensor.matmul(
                                    po[:, sl],
                                    vs[kb][:, kb % 2, h * 128:(h + 1) * 128],
                                    pb, start=(kb == 0), stop=(kb == 3))
                        rb = obp.tile([128, W2], f32, name="rb")
                        with nc.allow_low_precision("softmax denominator"):
                            nc.vector.reciprocal(rb[:], pd[:])
                        ot = otp.tile([128, W2], bf16, name="ot")
                        nc.vector.tensor_mul(ot[:], po[:], rb[:])
                        ots.append(ot)

                    # P3 for this chunk's 256 tokens
                    for tt in range(2):
                        for nt in range(4):
                            ps3 = pp3.tile([128, 512], f32, name="pp3")
                            for h in range(HEADS):
                                hp, i = divmod(h, 2)
                                lhs = ots[hp][:, i * WIN + tt * 128:
                                              i * WIN + tt * 128 + 128]
                                nc.tensor.matmul(
                                    ps3[:], lhs,
                                    wo[:, h, nt * 512:(nt + 1) * 512],
                                    start=(h == 0), stop=(h == HEADS - 1))
                            so = so3p.tile([128, 512], f32, name="so")
                            nc.scalar.copy(so[:], ps3[:])
                            nc.scalar.dma_start(
                                OUT[c * WIN + tt * 128:
                                    c * WIN + (tt + 1) * 128,
                                    nt * 512:(nt + 1) * 512], so[:])
                    kt_prev, v_prev = kt_cur, v_cur
    return nc


def _host_inputs(hidden_states, Wq, Wk, Wv, Wo, T):
    """Build the 8 per-core input maps."""
    TH = T + WIN
    inv_freq = 1.0 / (THETA ** (np.arange(0, HD, 2, dtype=np.float32) / HD))

    qq = np.arange(WIN)[None, :]
    kk = np.arange(128)[:, None]
    tri23 = np.concatenate([(qq >= kk), (qq >= kk + 128)], 1).astype(
        ml_dtypes.bfloat16)
    onesm_f32 = np.ones((128, 128), np.float32)

    wq32, wk32, wv32 = (np.ascontiguousarray(w, np.float32)
                        for w in (Wq, Wk, Wv))
    wo_bf = np.ascontiguousarray(np.asarray(Wo).astype(ml_dtypes.bfloat16))
    in_maps = []
    for core in range(8):
        b, sh = divmod(core, NSH)
        t0 = sh * T
        hs = np.zeros((TH, DIMS), np.float32)
        lo = max(0, t0 - WIN)
        hs[WIN - (t0 - lo):] = hidden_states[b, lo:t0 + T]
        hT = np.ascontiguousarray(hs.T)

        pos = np.arange(t0 - WIN, t0 + T, dtype=np.float32)
        f = np.outer(inv_freq, pos)                      # [64, TH]
        cos = np.concatenate([np.cos(f), np.cos(f)], 0)  # [128, TH]
        sin = np.sin(f)
        sins = np.concatenate([-sin, sin], 0)
        cos4 = np.ascontiguousarray(
            np.broadcast_to(cos[:, None, :], (128, 4, TH)).astype(
                ml_dtypes.bfloat16))
        sin4 = np.ascontiguousarray(
            np.broadcast_to(sins[:, None, :], (128, 4, TH)).astype(
                ml_dtypes.bfloat16))
        pg = np.full((128, 1), -1e30 if sh == 0 else 0.0, np.float32)
        in_maps.append({
            "HT": hT, "WQ": wq32, "WK": wk32, "WV": wv32, "WO": wo_bf,
            "COS4": cos4, "SIN4": sin4,
            "TRI23": tri23, "PGATE": pg, "ONESM": onesm_f32,
        })
    return in_maps


_CACHE = {}


def run(hidden_states, Wq, Wk, Wv, Wo, T=S // NSH, **spmd_kwargs):
    key = T
    if key not in _CACHE:
        nc = bacc.Bacc(None)
        build(nc, T)
        nc.finalize()
        _CACHE[key] = nc
    nc = _CACHE[key]
    in_maps = _host_inputs(hidden_states, Wq, Wk, Wv, Wo, T)
    res = run_bass_kernel_spmd(nc, in_maps, core_ids=list(range(8)),
                               **spmd_kwargs)
    outs = [res.results[i]["OUT"] for i in range(8)]
    full = np.empty((B, NSH * T, DIMS), np.float32)
    for core in range(8):
        b, sh = divmod(core, NSH)
        full[b, sh * T:(sh + 1) * T] = outs[core]
    return full, res


def kernel(hidden_states, Wq, Wk, Wv, Wo):
    out, _ = run(np.asarray(hidden_states), Wq, Wk, Wv, Wo)
    return out


# revision 26
# speedup vs baseline: 1.1094x; 1.0038x over previous
"""Block sliding-window attention on 8 TRN2 NeuronCores.

Sharding: sequence-parallel. 8 shards = (batch b in {0,1}) x (quarter s in
0..3); each core owns 2048 consecutive tokens of one batch plus a 256-token
K/V halo from the previous quarter (zeros + -inf gate for the first quarter).
No collectives: each core computes its tokens' full output rows.

Engine-cost notes driving the layout: every bf16-stationary matmul emits a
separate Ldweights (~146ns serial PE.SEQ); f32r stationaries self-load.
Walrus forbids mixing 16/32-bit matmul operands, and PSUM matmul outputs are
capped at one bank (512 f32). DMA instructions serialize ~625ns on HWDGE, so
transfers are batched into few big copies.

  P1 (all f32r, zero Ldweights): V = hidden @ Wv (256-col Wv panels),
      KT/QT = W^T @ hiddenT per head, staged to DRAM scratch bf16.
  P2+P3 fused per 256-token chunk: RoPE on Q/K (DVE, bf16), per head:
      S^T = K Q^T (bf16), exp on ACT -> f32r probs (pgate bias gates chunk
      0's no-previous case), triangular mask mul in-place on DVE,
      denominator pre-add on Pool + ones-matmul (f32r), O^T = V^T P^T
      (f32r; V cast bf16->f32r via gpsimd casting DMA), normalize via DVE
      reciprocal+mul, then out[chunk] = sum_h O_h @ Wo_h (bf16, resident
      Wo) accumulated in PSUM and stored straight to OUT.
"""
import sys

try:
    import concourse  # noqa: F401
except ImportError:
    sys.path.insert(0, '/opt/trn_rl_repo')

import ml_dtypes
import numpy as np

import concourse.bacc as bacc
import concourse.mybir as mybir
import concourse.tile as tile
from concourse.bass_utils import run_bass_kernel_spmd

f32 = mybir.dt.float32
f32r = mybir.dt.float32r
bf16 = mybir.dt.bfloat16
AF = mybir.ActivationFunctionType

DIMS = 2048
HEADS = 16
HD = 128           # head dim
WIN = 256          # window / chunk
B, S = 2, 8192
NSH = 4            # seq shards per batch
THETA = 10000.0
ISQ = float(1.0 / np.sqrt(HD))
IB = DIMS // 128   # 16 input-dim blocks


def tok_tiles(n):
    out, a = [], 0
    while a < n:
        w = min(512, n - a)
        out.append((a, w))
        a += w
    return out


def build(nc, T):
    """Emit the per-core program. T = local tokens (multiple of 512)."""
    TH = T + WIN                      # with halo
    NC_ = T // WIN                    # chunks
    NTB = TH // 128                   # 128-token blocks incl halo
    HT = nc.dram_tensor("HT", [DIMS, TH], f32r, kind="ExternalInput")
    WQ = nc.dram_tensor("WQ", [DIMS, DIMS], f32r, kind="ExternalInput")
    WK = nc.dram_tensor("WK", [DIMS, DIMS], f32r, kind="ExternalInput")
    WV = nc.dram_tensor("WV", [DIMS, DIMS], f32r, kind="ExternalInput")
    WO = nc.dram_tensor("WO", [DIMS, DIMS], bf16, kind="ExternalInput")
    COS4 = nc.dram_tensor("COS4", [128, 4, TH], bf16, kind="ExternalInput")
    SIN4 = nc.dram_tensor("SIN4", [128, 4, TH], bf16, kind="ExternalInput")
    TRI23 = nc.dram_tensor("TRI23", [128, 2 * WIN], bf16, kind="ExternalInput")
    PGATE = nc.dram_tensor("PGATE", [128, 1], f32, kind="ExternalInput")
    ONESM = nc.dram_tensor("ONESM", [128, 128], f32r, kind="ExternalInput")
    OUT = nc.dram_tensor("OUT", [T, DIMS], f32, kind="ExternalOutput")

    QTS = nc.dram_tensor("QTS", [HEADS, HD, T], bf16)    # raw (pre-RoPE) Q^T
    KTS = nc.dram_tensor("KTS", [HEADS, HD, TH], bf16)   # raw K^T (with halo)
    VS = nc.dram_tensor("VS", [TH, DIMS], bf16)          # V natural

    with tile.TileContext(nc) as tc:
        with tc.tile_pool(name="cst", bufs=1) as cst:
            tri23 = cst.tile([128, 2 * WIN], bf16)
            pgate = cst.tile([128, 1], f32)
            onesm = cst.tile([128, 128], f32r)
            nc.sync.dma_start(tri23[:], TRI23[:])
            nc.sync.dma_start(pgate[:], PGATE[:])
            nc.sync.dma_start(onesm[:], ONESM[:])

            # ---------------- P1: projections (all f32r) ----------------
            with tc.tile_pool(name="p1", bufs=1) as p1:
                ht = p1.tile([128, IB, TH], f32r)
                htr = HT.rearrange("(ib p) t -> p ib t", p=128)
                NG = 16
                tg = TH // NG
                nc.sync.dma_start(ht[:, :, 0:tg], htr[:, :, 0:tg])

                # V natural: lhsT = hT block [128in, 128tok], rhs = Wv panel
                with tc.tile_pool(name="wv", bufs=2) as wvp, \
                     tc.tile_pool(name="vb", bufs=2) as vbp, \
                     tc.tile_pool(name="vps", bufs=8, space="PSUM") as vps:
                    wvr = WV.rearrange("(ib p) o -> p ib o", p=128)
                    vsr = VS.rearrange("(tb p) c -> p tb c", p=128)
                    for og in range(8):
                        wv = wvp.tile([128, IB, 256], f32r, name="wv")
                        nc.sync.dma_start(
                            wv[:], wvr[:, :, og * 256:(og + 1) * 256])
                        if og == 0:
                            for g in range(1, NG):
                                nc.sync.dma_start(
                                    ht[:, :, g * tg:(g + 1) * tg],
                                    htr[:, :, g * tg:(g + 1) * tg])
                        vbog = vbp.tile([128, NTB, 256], bf16, name="vb")
                        for tb in range(NTB):
                            ps = vps.tile([128, 256], f32, name="vps")
                            for ib in range(IB):
                                nc.tensor.matmul(
                                    ps[:], ht[:, ib, tb * 128:(tb + 1) * 128],
                                    wv[:, ib, :],
                                    start=(ib == 0), stop=(ib == IB - 1))
                            nc.vector.tensor_copy(vbog[:, tb, :], ps[:])
                        nc.scalar.dma_start(
                            vsr[:, :, og * 256:(og + 1) * 256], vbog[:])

                # KT then QT: lhsT = W block [128in, 128out], rhs = hT
                with tc.tile_pool(name="wkq", bufs=3) as wkqp, \
                     tc.tile_pool(name="kst", bufs=3) as kstp, \
                     tc.tile_pool(name="kqps", bufs=8, space="PSUM") as kqps:
                    for W_, DST, t0, tlen in ((WK, KTS, 0, TH),
                                              (WQ, QTS, WIN, T)):
                        wr = W_.rearrange("(ib p) o -> p ib o", p=128)
                        tts = tok_tiles(tlen)
                        for ob in range(HEADS):
                            wt = wkqp.tile([128, IB, 128], f32r, name="wkq")
                            nc.sync.dma_start(
                                wt[:], wr[:, :, ob * 128:(ob + 1) * 128])
                            st = kstp.tile([128, TH], bf16, name="kst")
                            psums = [kqps.tile([128, 512], f32, name="kqps")
                                     for _ in tts]
                            for ib in range(IB):
                                for ti, (a, w) in enumerate(tts):
                                    nc.tensor.matmul(
                                        psums[ti][:, :w], wt[:, ib, :],
                                        ht[:, ib, t0 + a:t0 + a + w],
                                        start=(ib == 0), stop=(ib == IB - 1))
                            for ti, (a, w) in enumerate(tts):
                                nc.scalar.copy(st[:, a:a + w], psums[ti][:, :w])
                            nc.scalar.dma_start(DST[ob][:, :], st[:, :tlen])

            # ---------------- P2 + P3 fused ----------------
            with tc.tile_pool(name="wop", bufs=1) as wop, \
                 tc.tile_pool(name="qk", bufs=2) as qk, \
                 tc.tile_pool(name="rt", bufs=2) as rt, \
                 tc.tile_pool(name="tp", bufs=1) as tp, \
                 tc.tile_pool(name="ptp", bufs=2) as ptp, \
                 tc.tile_pool(name="pa", bufs=2) as pap, \
                 tc.tile_pool(name="ob", bufs=2) as obp, \
                 tc.tile_pool(name="ot", bufs=9) as otp, \
                 tc.tile_pool(name="so3", bufs=2) as so3p, \
                 tc.tile_pool(name="ps_s", bufs=2, space="PSUM") as ps_s, \
                 tc.tile_pool(name="ps_d", bufs=2, space="PSUM") as ps_d, \
                 tc.tile_pool(name="ps_o", bufs=2, space="PSUM") as ps_o, \
                 tc.tile_pool(name="pp3", bufs=2, space="PSUM") as pp3:
                wo = wop.tile([128, IB, DIMS], bf16)
                wor = WO.rearrange("(ib p) o -> p ib o", p=128)

                def rope_load(SRC, c0, roped, pos0=None):
                    """Load [128, HEADS, WIN] token window at c0 from SRC
                    (head-major scratch), apply RoPE into `roped` (bf16).
                    pos0: column into COS4/SIN4 (halo coords); default c0.
                    DMAs go out per 4-head group on the DVE hwdge queue so
                    each group's rope pipelines with the producer stores."""
                    if pos0 is None:
                        pos0 = c0
                    raw = rt.tile([128, HEADS, WIN], bf16, name="raw")
                    rot = rt.tile([128, HEADS, WIN], bf16, name="rot")
                    cos4 = tp.tile([128, 4, WIN], bf16, name="cos4")
                    sin4 = tp.tile([128, 4, WIN], bf16, name="sin4")
                    nc.sync.dma_start(cos4[:], COS4[:, :, pos0:pos0 + WIN])
                    nc.sync.dma_start(sin4[:], SIN4[:, :, pos0:pos0 + WIN])
                    sl = SRC[:, :, c0:c0 + WIN]
                    for g in range(4):
                        gs = slice(g * 4, (g + 1) * 4)
                        sg = sl[g * 4:(g + 1) * 4]
                        nc.sync.dma_start(
                            raw[:, gs], sg.rearrange("h d w -> d h w"))
                        nc.sync.dma_start(
                            rot[0:64, gs],
                            sg[:, 64:128, :].rearrange("h d w -> d h w"))
                        nc.sync.dma_start(
                            rot[64:128, gs],
                            sg[:, 0:64, :].rearrange("h d w -> d h w"))
                        tmp = tp.tile([128, 4, WIN], bf16, name="tmp")
                        nc.vector.tensor_mul(tmp[:], rot[:, gs], sin4[:])
                        nc.vector.tensor_mul(roped[:, gs], raw[:, gs], cos4[:])
                        nc.vector.tensor_add(roped[:, gs], roped[:, gs], tmp[:])

                kt_prev = qk.tile([128, HEADS, WIN], bf16, name="kt")
                rope_load(KTS, 0, kt_prev)
                v_prev = qk.tile([128, 2, DIMS], f32r, name="v")
                nc.gpsimd.dma_start(
                    v_prev[:], VS[0:WIN].rearrange("(tb p) c -> p tb c", p=128))

                W2 = 2 * WIN
                for c in range(NC_):
                    qt = qk.tile([128, HEADS, WIN], bf16, name="qt")
                    rope_load(QTS, c * WIN, qt, pos0=WIN + c * WIN)
                    v_cur = qk.tile([128, 2, DIMS], f32r, name="v")
                    nc.gpsimd.dma_start(
                        v_cur[:], VS[WIN + c * WIN:WIN + (c + 1) * WIN]
                        .rearrange("(tb p) c -> p tb c", p=128))
                    kt_cur = qk.tile([128, HEADS, WIN], bf16, name="kt")
                    rope_load(KTS, WIN + c * WIN, kt_cur)
                    if c == 0:
                        # wo quarters, emitted after chunk-0 prep so its bulk
                        # doesn't delay the seam-critical rope loads
                        for g in range(4):
                            nc.sync.dma_start(
                                wo[:, :, g * 512:(g + 1) * 512],
                                wor[:, :, g * 512:(g + 1) * 512])

                    kts = [kt_prev, kt_prev, kt_cur, kt_cur]
                    vs = [v_prev, v_prev, v_cur, v_cur]
                    ots = []
                    for h0 in range(0, HEADS, 2):
                        pd = ps_d.tile([128, W2], f32, name="pd")
                        po = ps_o.tile([128, W2], f32, name="po")
                        pbs2, pads = [], []
                        for h in (h0, h0 + 1):
                            pbs = []
                            for pr in range(2):
                                ps = ps_s.tile([128, W2], f32, name="ps")
                                for kb2 in range(2):
                                    kb = pr * 2 + kb2
                                    nc.tensor.matmul(
                                        ps[:, kb2 * WIN:(kb2 + 1) * WIN],
                                        kts[kb][:, h,
                                                (kb % 2) * 128:(kb % 2) * 128 + 128],
                                        qt[:, h], start=True, stop=True)
                                pb = ptp.tile([128, W2], f32r, name=f"pt{pr}")
                                if pr == 0:
                                    if c == 0:
                                        nc.scalar.activation(
                                            pb[:], ps[:], AF.Exp,
                                            bias=pgate[:], scale=ISQ)
                                    else:
                                        nc.scalar.activation(
                                            pb[:], ps[:], AF.Exp, scale=ISQ)
                                else:
                                    nc.scalar.activation(pb[:], ps[:], AF.Exp,
                                                         scale=ISQ)
                                    nc.vector.tensor_mul(pb[:], pb[:],
                                                         tri23[:])
                                pbs.append(pb)
                            pbs2.append(pbs)
                            pad = pap.tile([128, W2], f32r, name="pad")
                            nc.gpsimd.tensor_add(pad[:], pbs[0][:], pbs[1][:])
                            pads.append(pad)

                        for i, h in enumerate((h0, h0 + 1)):
                            sl = slice(i * WIN, (i + 1) * WIN)
                            for half in range(2):
                                nc.tensor.matmul(
                                    pd[:, sl], onesm[:],
                                    pads[i][:, half * WIN:(half + 1) * WIN],
                                    start=(half == 0), stop=(half == 1))
                            for kb in range(4):
                                pb = pbs2[i][kb // 2][
                                    :, (kb % 2) * WIN:(kb % 2 + 1) * WIN]
                                nc.tensor.matmul(
                                    po[:, sl],
                                    vs[kb][:, kb % 2, h * 128:(h + 1) * 128],
                                    pb, start=(kb == 0), stop=(kb == 3))
                        rb = obp.tile([128, W2], f32, name="rb")
                        with nc.allow_low_precision("softmax denominator"):
                            nc.vector.reciprocal(rb[:], pd[:])
                        ot = otp.tile([128, W2], bf16, name="ot")
                        nc.vector.tensor_mul(ot[:], po[:], rb[:])
                        ots.append(ot)

                    # P3 for this chunk's 256 tokens
                    for tt in range(2):
                        for nt in range(4):
                            ps3 = pp3.tile([128, 512], f32, name="pp3")
                            for h in range(HEADS):
                                hp, i = divmod(h, 2)
                                lhs = ots[hp][:, i * WIN + tt * 128:
                                              i * WIN + tt * 128 + 128]
                                nc.tensor.matmul(
                                    ps3[:], lhs,
                                    wo[:, h, nt * 512:(nt + 1) * 512],
                                    start=(h == 0), stop=(h == HEADS - 1))
                            so = so3p.tile([128, 512], f32, name="so")
                            nc.scalar.copy(so[:], ps3[:])
                            nc.scalar.dma_start(
                                OUT[c * WIN + tt * 128:
                                    c * WIN + (tt + 1) * 128,
                                    nt * 512:(nt + 1) * 512], so[:])
                    kt_prev, v_prev = kt_cur, v_cur
    return nc


def _host_inputs(hidden_states, Wq, Wk, Wv, Wo, T):
    """Build the 8 per-core input maps."""
    TH = T + WIN
    inv_freq = 1.0 / (THETA ** (np.arange(0, HD, 2, dtype=np.float32) / HD))

    qq = np.arange(WIN)[None, :]
    kk = np.arange(128)[:, None]
    tri23 = np.concatenate([(qq >= kk), (qq >= kk + 128)], 1).astype(
        ml_dtypes.bfloat16)
    onesm_f32 = np.ones((128, 128), np.float32)

    wq32, wk32, wv32 = (np.ascontiguousarray(w, np.float32)
                        for w in (Wq, Wk, Wv))
    wo_bf = np.ascontiguousarray(np.asarray(Wo).astype(ml_dtypes.bfloat16))
    in_maps = []
    for core in range(8):
        b, sh = divmod(core, NSH)
        t0 = sh * T
        hs = np.zeros((TH, DIMS), np.float32)
        lo = max(0, t0 - WIN)
        hs[WIN - (t0 - lo):] = hidden_states[b, lo:t0 + T]
        hT = np.ascontiguousarray(hs.T)

        pos = np.arange(t0 - WIN, t0 + T, dtype=np.float32)
        f = np.outer(inv_freq, pos)                      # [64, TH]
        cos = np.concatenate([np.cos(f), np.cos(f)], 0)  # [128, TH]
        sin = np.sin(f)
        sins = np.concatenate([-sin, sin], 0)
        cos4 = np.ascontiguousarray(
            np.broadcast_to(cos[:, None, :], (128, 4, TH)).astype(
                ml_dtypes.bfloat16))
        sin4 = np.ascontiguousarray(
            np.broadcast_to(sins[:, None, :], (128, 4, TH)).astype(
                ml_dtypes.bfloat16))
        pg = np.full((128, 1), -1e30 if sh == 0 else 0.0, np.float32)
        in_maps.append({
            "HT": hT, "WQ": wq32, "WK": wk32, "WV": wv32, "WO": wo_bf,
            "COS4": cos4, "SIN4": sin4,
            "TRI23": tri23, "PGATE": pg, "ONESM": onesm_f32,
        })
    return in_maps


_CACHE = {}


def run(hidden_states, Wq, Wk, Wv, Wo, T=S // NSH, **spmd_kwargs):
    key = T
    if key not in _CACHE:
        nc = bacc.Bacc(None)
        build(nc, T)
        nc.finalize()
        _CACHE[key] = nc
    nc = _CACHE[key]
    in_maps = _host_inputs(hidden_states, Wq, Wk, Wv, Wo, T)
    res = run_bass_kernel_spmd(nc, in_maps, core_ids=list(range(8)),
                               **spmd_kwargs)
    outs = [res.results[i]["OUT"] for i in range(8)]
    full = np.empty((B, NSH * T, DIMS), np.float32)
    for core in range(8):
        b, sh = divmod(core, NSH)
        full[b, sh * T:(sh + 1) * T] = outs[core]
    return full, res


def kernel(hidden_states, Wq, Wk, Wv, Wo):
    out, _ = run(np.asarray(hidden_states), Wq, Wk, Wv, Wo)
    return out


# revision 28
# speedup vs baseline: 1.1292x; 1.0179x over previous
"""Block sliding-window attention on 8 TRN2 NeuronCores.

Sharding: sequence-parallel. 8 shards = (batch b in {0,1}) x (quarter s in
0..3); each core owns 2048 consecutive tokens of one batch plus a 256-token
K/V halo from the previous quarter (zeros + -inf gate for the first quarter).
No collectives: each core computes its tokens' full output rows.

Engine-cost notes driving the layout: every bf16-stationary matmul emits a
separate Ldweights (~146ns serial PE.SEQ); f32r stationaries self-load.
Walrus forbids mixing 16/32-bit matmul operands, and PSUM matmul outputs are
capped at one bank (512 f32). DMA instructions serialize ~625ns on HWDGE, so
transfers are batched into few big copies.

  P1 (all f32r, zero Ldweights): V = hidden @ Wv (256-col Wv panels),
      KT/QT = W^T @ hiddenT per head, staged to DRAM scratch bf16.
  P2+P3 fused per 256-token chunk: RoPE on Q/K (DVE, bf16), per head:
      S^T = K Q^T (bf16), exp on ACT -> f32r probs (pgate bias gates chunk
      0's no-previous case), triangular mask mul in-place on DVE,
      denominator pre-add on Pool + ones-matmul (f32r), O^T = V^T P^T
      (f32r; V cast bf16->f32r via gpsimd casting DMA), normalize via DVE
      reciprocal+mul, then out[chunk] = sum_h O_h @ Wo_h (bf16, resident
      Wo) accumulated in PSUM and stored straight to OUT.
"""
import sys

try:
    import concourse  # noqa: F401
except ImportError:
    sys.path.insert(0, '/opt/trn_rl_repo')

import ml_dtypes
import numpy as np

import concourse.bacc as bacc
import concourse.mybir as mybir
import concourse.tile as tile
from concourse.bass_utils import run_bass_kernel_spmd

f32 = mybir.dt.float32
f32r = mybir.dt.float32r
bf16 = mybir.dt.bfloat16
AF = mybir.ActivationFunctionType

DIMS = 2048
HEADS = 16
HD = 128           # head dim
WIN = 256          # window / chunk
B, S = 2, 8192
NSH = 4            # seq shards per batch
THETA = 10000.0
ISQ = float(1.0 / np.sqrt(HD))
IB = DIMS // 128   # 16 input-dim blocks


def tok_tiles(n):
    out, a = [], 0
    while a < n:
        w = min(512, n - a)
        out.append((a, w))
        a += w
    return out


def build(nc, T):
    """Emit the per-core program. T = local tokens (multiple of 512)."""
    TH = T + WIN                      # with halo
    NC_ = T // WIN                    # chunks
    NTB = TH // 128                   # 128-token blocks incl halo
    HT = nc.dram_tensor("HT", [DIMS, TH], f32r, kind="ExternalInput")
    WQ = nc.dram_tensor("WQ", [DIMS, DIMS], f32r, kind="ExternalInput")
    WK = nc.dram_tensor("WK", [DIMS, DIMS], f32r, kind="ExternalInput")
    WV = nc.dram_tensor("WV", [DIMS, DIMS], f32r, kind="ExternalInput")
    WO = nc.dram_tensor("WO", [DIMS, DIMS], bf16, kind="ExternalInput")
    COS4 = nc.dram_tensor("COS4", [128, 4, TH], bf16, kind="ExternalInput")
    SIN4 = nc.dram_tensor("SIN4", [128, 4, TH], bf16, kind="ExternalInput")
    TRI23 = nc.dram_tensor("TRI23", [128, 2 * WIN], bf16, kind="ExternalInput")
    PGATE = nc.dram_tensor("PGATE", [128, 1], f32, kind="ExternalInput")
    ONESM = nc.dram_tensor("ONESM", [128, 128], f32r, kind="ExternalInput")
    OUT = nc.dram_tensor("OUT", [T, DIMS], f32, kind="ExternalOutput")

    QTS = nc.dram_tensor("QTS", [HEADS, HD, T], bf16)    # raw (pre-RoPE) Q^T
    KTS = nc.dram_tensor("KTS", [HEADS, HD, TH], bf16)   # raw K^T (with halo)
    VS = nc.dram_tensor("VS", [TH, DIMS], bf16)          # V natural

    with tile.TileContext(nc) as tc:
        with tc.tile_pool(name="cst", bufs=1) as cst:
            tri23 = cst.tile([128, 2 * WIN], bf16)
            pgate = cst.tile([128, 1], f32)
            onesm = cst.tile([128, 128], f32r)
            nc.sync.dma_start(tri23[:], TRI23[:])
            nc.sync.dma_start(pgate[:], PGATE[:])
            nc.sync.dma_start(onesm[:], ONESM[:])

            # ---------------- P1: projections (all f32r) ----------------
            with tc.tile_pool(name="p1", bufs=1) as p1:
                ht = p1.tile([128, IB, TH], f32r)
                htr = HT.rearrange("(ib p) t -> p ib t", p=128)
                NG = 16
                tg = TH // NG
                nc.sync.dma_start(ht[:, :, 0:tg], htr[:, :, 0:tg])

                # V natural: lhsT = hT block [128in, 128tok], rhs = Wv panel
                with tc.tile_pool(name="wv", bufs=2) as wvp, \
                     tc.tile_pool(name="vb", bufs=2) as vbp, \
                     tc.tile_pool(name="vps", bufs=8, space="PSUM") as vps:
                    wvr = WV.rearrange("(ib p) o -> p ib o", p=128)
                    vsr = VS.rearrange("(tb p) c -> p tb c", p=128)
                    for og in range(8):
                        wv = wvp.tile([128, IB, 256], f32r, name="wv")
                        nc.sync.dma_start(
                            wv[:], wvr[:, :, og * 256:(og + 1) * 256])
                        if og == 0:
                            for g in range(1, NG):
                                nc.sync.dma_start(
                                    ht[:, :, g * tg:(g + 1) * tg],
                                    htr[:, :, g * tg:(g + 1) * tg])
                        vbog = vbp.tile([128, NTB, 256], bf16, name="vb")
                        for tb in range(NTB):
                            ps = vps.tile([128, 256], f32, name="vps")
                            for ib in range(IB):
                                nc.tensor.matmul(
                                    ps[:], ht[:, ib, tb * 128:(tb + 1) * 128],
                                    wv[:, ib, :],
                                    start=(ib == 0), stop=(ib == IB - 1))
                            nc.vector.tensor_copy(vbog[:, tb, :], ps[:])
                        nc.scalar.dma_start(
                            vsr[:, :, og * 256:(og + 1) * 256], vbog[:])

                # KT then QT: lhsT = W block [128in, 128out], rhs = hT
                with tc.tile_pool(name="wkq", bufs=3) as wkqp, \
                     tc.tile_pool(name="kst", bufs=3) as kstp, \
                     tc.tile_pool(name="kqps", bufs=8, space="PSUM") as kqps:
                    for W_, DST, t0, tlen in ((WK, KTS, 0, TH),
                                              (WQ, QTS, WIN, T)):
                        wr = W_.rearrange("(ib p) o -> p ib o", p=128)
                        tts = tok_tiles(tlen)
                        for ob in range(HEADS):
                            wt = wkqp.tile([128, IB, 128], f32r, name="wkq")
                            nc.sync.dma_start(
                                wt[:], wr[:, :, ob * 128:(ob + 1) * 128])
                            st = kstp.tile([128, TH], bf16, name="kst")
                            psums = [kqps.tile([128, 512], f32, name="kqps")
                                     for _ in tts]
                            for ib in range(IB):
                                for ti, (a, w) in enumerate(tts):
                                    nc.tensor.matmul(
                                        psums[ti][:, :w], wt[:, ib, :],
                                        ht[:, ib, t0 + a:t0 + a + w],
                                        start=(ib == 0), stop=(ib == IB - 1))
                            for ti, (a, w) in enumerate(tts):
                                nc.scalar.copy(st[:, a:a + w], psums[ti][:, :w])
                            nc.scalar.dma_start(DST[ob][:, :], st[:, :tlen])

            # ---------------- P2 + P3 fused ----------------
            with tc.tile_pool(name="wop", bufs=1) as wop, \
                 tc.tile_pool(name="qk", bufs=2) as qk, \
                 tc.tile_pool(name="rt", bufs=2) as rt, \
                 tc.tile_pool(name="tp", bufs=1) as tp, \
                 tc.tile_pool(name="ptp", bufs=2) as ptp, \
                 tc.tile_pool(name="pa", bufs=2) as pap, \
                 tc.tile_pool(name="ob", bufs=2) as obp, \
                 tc.tile_pool(name="ot", bufs=9) as otp, \
                 tc.tile_pool(name="so3", bufs=2) as so3p, \
                 tc.tile_pool(name="ps_s", bufs=2, space="PSUM") as ps_s, \
                 tc.tile_pool(name="ps_d", bufs=2, space="PSUM") as ps_d, \
                 tc.tile_pool(name="ps_o", bufs=2, space="PSUM") as ps_o, \
                 tc.tile_pool(name="pp3", bufs=2, space="PSUM") as pp3:
                wo = wop.tile([128, IB, DIMS], bf16)
                wor = WO.rearrange("(ib p) o -> p ib o", p=128)

                def rope_load(SRC, c0, roped, pos0=None):
                    """Load [128, HEADS, WIN] token window at c0 from SRC
                    (head-major scratch), apply RoPE into `roped` (bf16).
                    pos0: column into COS4/SIN4 (halo coords); default c0.
                    DMAs go out per 4-head group on the DVE hwdge queue so
                    each group's rope pipelines with the producer stores."""
                    if pos0 is None:
                        pos0 = c0
                    raw = rt.tile([128, HEADS, WIN], bf16, name="raw")
                    rot = rt.tile([128, HEADS, WIN], bf16, name="rot")
                    cos4 = tp.tile([128, 4, WIN], bf16, name="cos4")
                    sin4 = tp.tile([128, 4, WIN], bf16, name="sin4")
                    nc.sync.dma_start(cos4[:], COS4[:, :, pos0:pos0 + WIN])
                    nc.sync.dma_start(sin4[:], SIN4[:, :, pos0:pos0 + WIN])
                    sl = SRC[:, :, c0:c0 + WIN]
                    for g in range(4):
                        gs = slice(g * 4, (g + 1) * 4)
                        sg = sl[g * 4:(g + 1) * 4]
                        nc.sync.dma_start(
                            raw[:, gs], sg.rearrange("h d w -> d h w"))
                        nc.sync.dma_start(
                            rot[0:64, gs],
                            sg[:, 64:128, :].rearrange("h d w -> d h w"))
                        nc.sync.dma_start(
                            rot[64:128, gs],
                            sg[:, 0:64, :].rearrange("h d w -> d h w"))
                        tmp = tp.tile([128, 4, WIN], bf16, name="tmp")
                        nc.vector.tensor_mul(tmp[:], rot[:, gs], sin4[:])
                        nc.vector.tensor_mul(roped[:, gs], raw[:, gs], cos4[:])
                        nc.vector.tensor_add(roped[:, gs], roped[:, gs], tmp[:])

                def v_load(c0, vt):
                    """Cast-load [128, 2, DIMS] V window (bf16 -> f32r) in
                    4-head-group slices so early heads' PV never waits on
                    the whole 4MB transfer."""
                    src = VS[c0:c0 + WIN].rearrange("(tb p) c -> p tb c",
                                                    p=128)
                    for g in range(4):
                        cs = slice(g * 512, (g + 1) * 512)
                        nc.gpsimd.dma_start(vt[:, :, cs], src[:, :, cs])

                kt_prev = qk.tile([128, HEADS, WIN], bf16, name="kt")
                rope_load(KTS, 0, kt_prev)
                v_prev = qk.tile([128, 2, DIMS], f32r, name="v")
                v_load(0, v_prev)

                W2 = 2 * WIN
                for c in range(NC_):
                    qt = qk.tile([128, HEADS, WIN], bf16, name="qt")
                    rope_load(QTS, c * WIN, qt, pos0=WIN + c * WIN)
                    kt_cur = qk.tile([128, HEADS, WIN], bf16, name="kt")
                    rope_load(KTS, WIN + c * WIN, kt_cur)
                    v_cur = qk.tile([128, 2, DIMS], f32r, name="v")
                    v_load(WIN + c * WIN, v_cur)
                    if c == 0:
                        # wo quarters, emitted after chunk-0 prep so its bulk
                        # doesn't delay the seam-critical rope loads
                        for g in range(4):
                            nc.sync.dma_start(
                                wo[:, :, g * 512:(g + 1) * 512],
                                wor[:, :, g * 512:(g + 1) * 512])

                    kts = [kt_prev, kt_prev, kt_cur, kt_cur]
                    vs = [v_prev, v_prev, v_cur, v_cur]
                    ots = []
                    for h0 in range(0, HEADS, 2):
                        pd = ps_d.tile([128, W2], f32, name="pd")
                        po = ps_o.tile([128, W2], f32, name="po")
                        pbs2, pads = [], []
                        for h in (h0, h0 + 1):
                            pbs = []
                            for pr in range(2):
                                ps = ps_s.tile([128, W2], f32, name="ps")
                                for kb2 in range(2):
                                    kb = pr * 2 + kb2
                                    nc.tensor.matmul(
                                        ps[:, kb2 * WIN:(kb2 + 1) * WIN],
                                        kts[kb][:, h,
                                                (kb % 2) * 128:(kb % 2) * 128 + 128],
                                        qt[:, h], start=True, stop=True)
                                pb = ptp.tile([128, W2], f32r, name=f"pt{pr}")
                                if pr == 0:
                                    if c == 0:
                                        nc.scalar.activation(
                                            pb[:], ps[:], AF.Exp,
                                            bias=pgate[:], scale=ISQ)
                                    else:
                                        nc.scalar.activation(
                                            pb[:], ps[:], AF.Exp, scale=ISQ)
                                else:
                                    nc.scalar.activation(pb[:], ps[:], AF.Exp,
                                                         scale=ISQ)
                                    nc.vector.tensor_mul(pb[:], pb[:],
                                                         tri23[:])
                                pbs.append(pb)
                            pbs2.append(pbs)
                            pad = pap.tile([128, W2], f32r, name="pad")
                            nc.gpsimd.tensor_add(pad[:], pbs[0][:], pbs[1][:])
                            pads.append(pad)

                        for i, h in enumerate((h0, h0 + 1)):
                            sl = slice(i * WIN, (i + 1) * WIN)
                            for half in range(2):
                                nc.tensor.matmul(
                                    pd[:, sl], onesm[:],
                                    pads[i][:, half * WIN:(half + 1) * WIN],
                                    start=(half == 0), stop=(half == 1))
                            for kb in range(4):
                                pb = pbs2[i][kb // 2][
                                    :, (kb % 2) * WIN:(kb % 2 + 1) * WIN]
                                nc.tensor.matmul(
                                    po[:, sl],
                                    vs[kb][:, kb % 2, h * 128:(h + 1) * 128],
                                    pb, start=(kb == 0), stop=(kb == 3))
                        rb = obp.tile([128, W2], f32, name="rb")
                        with nc.allow_low_precision("softmax denominator"):
                            nc.vector.reciprocal(rb[:], pd[:])
                        ot = otp.tile([128, W2], bf16, name="ot")
                        nc.vector.tensor_mul(ot[:], po[:], rb[:])
                        ots.append(ot)

                    # P3 for this chunk's 256 tokens
                    for tt in range(2):
                        for nt in range(4):
                            ps3 = pp3.tile([128, 512], f32, name="pp3")
                            for h in range(HEADS):
                                hp, i = divmod(h, 2)
                                lhs = ots[hp][:, i * WIN + tt * 128:
                                              i * WIN + tt * 128 + 128]
                                nc.tensor.matmul(
                                    ps3[:], lhs,
                                    wo[:, h, nt * 512:(nt + 1) * 512],
                                    start=(h == 0), stop=(h == HEADS - 1))
                            so = so3p.tile([128, 512], f32, name="so")
                            nc.scalar.copy(so[:], ps3[:])
                            nc.scalar.dma_start(
                                OUT[c * WIN + tt * 128:
                                    c * WIN + (tt + 1) * 128,
                                    nt * 512:(nt + 1) * 512], so[:])
                    kt_prev, v_prev = kt_cur, v_cur
    return nc


def _host_inputs(hidden_states, Wq, Wk, Wv, Wo, T):
    """Build the 8 per-core input maps."""
    TH = T + WIN
    inv_freq = 1.0 / (THETA ** (np.arange(0, HD, 2, dtype=np.float32) / HD))

    qq = np.arange(WIN)[None, :]
    kk = np.arange(128)[:, None]
    tri23 = np.concatenate([(qq >= kk), (qq >= kk + 128)], 1).astype(
        ml_dtypes.bfloat16)
    onesm_f32 = np.ones((128, 128), np.float32)

    wq32, wk32, wv32 = (np.ascontiguousarray(w, np.float32)
                        for w in (Wq, Wk, Wv))
    wo_bf = np.ascontiguousarray(np.asarray(Wo).astype(ml_dtypes.bfloat16))
    in_maps = []
    for core in range(8):
        b, sh = divmod(core, NSH)
        t0 = sh * T
        hs = np.zeros((TH, DIMS), np.float32)
        lo = max(0, t0 - WIN)
        hs[WIN - (t0 - lo):] = hidden_states[b, lo:t0 + T]
        hT = np.ascontiguousarray(hs.T)

        pos = np.arange(t0 - WIN, t0 + T, dtype=np.float32)
        f = np.outer(inv_freq, pos)                      # [64, TH]
        cos = np.concatenate([np.cos(f), np.cos(f)], 0)  # [128, TH]
        sin = np.sin(f)
        sins = np.concatenate([-sin, sin], 0)
        cos4 = np.ascontiguousarray(
            np.broadcast_to(cos[:, None, :], (128, 4, TH)).astype(
                ml_dtypes.bfloat16))
        sin4 = np.ascontiguousarray(
            np.broadcast_to(sins[:, None, :], (128, 4, TH)).astype(
                ml_dtypes.bfloat16))
        pg = np.full((128, 1), -1e30 if sh == 0 else 0.0, np.float32)
        in_maps.append({
            "HT": hT, "WQ": wq32, "WK": wk32, "WV": wv32, "WO": wo_bf,
            "COS4": cos4, "SIN4": sin4,
            "TRI23": tri23, "PGATE": pg, "ONESM": onesm_f32,
        })
    return in_maps


_CACHE = {}


def run(hidden_states, Wq, Wk, Wv, Wo, T=S // NSH, **spmd_kwargs):
    key = T
    if key not in _CACHE:
        nc = bacc.Bacc(None)
        build(nc, T)
        nc.finalize()
        _CACHE[key] = nc
    nc = _CACHE[key]
    in_maps = _host_inputs(hidden_states, Wq, Wk, Wv, Wo, T)
    res = run_bass_kernel_spmd(nc, in_maps, core_ids=list(range(8)),
                               **spmd_kwargs)
    outs = [res.results[i]["OUT"] for i in range(8)]
    full = np.empty((B, NSH * T, DIMS), np.float32)
    for core in range(8):
        b, sh = divmod(core, NSH)
        full[b, sh * T:(sh + 1) * T] = outs[core]
    return full, res


def kernel(hidden_states, Wq, Wk, Wv, Wo):
    out, _ = run(np.asarray(hidden_states), Wq, Wk, Wv, Wo)
    return out


# revision 33
# speedup vs baseline: 1.1342x; 1.0045x over previous
"""Block sliding-window attention on 8 TRN2 NeuronCores.

Sharding: sequence-parallel. 8 shards = (batch b in {0,1}) x (quarter s in
0..3); each core owns 2048 consecutive tokens of one batch plus a 256-token
K/V halo from the previous quarter (zeros + -inf gate for the first quarter).
No collectives: each core computes its tokens' full output rows.

Engine-cost notes driving the layout: every bf16-stationary matmul emits a
separate Ldweights (~146ns serial PE.SEQ); f32r stationaries self-load.
Walrus forbids mixing 16/32-bit matmul operands, and PSUM matmul outputs are
capped at one bank (512 f32). DMA instructions serialize ~625ns on HWDGE, so
transfers are batched into few big copies.

  P1 (all f32r, zero Ldweights): V = hidden @ Wv (256-col Wv panels),
      KT/QT = W^T @ hiddenT per head, staged to DRAM scratch bf16.
  P2+P3 fused per 256-token chunk: RoPE on Q/K (DVE, bf16), per head:
      S^T = K Q^T (bf16), exp on ACT -> f32r probs (pgate bias gates chunk
      0's no-previous case), triangular mask mul in-place on DVE,
      denominator pre-add on Pool + ones-matmul (f32r), O^T = V^T P^T
      (f32r; V cast bf16->f32r via gpsimd casting DMA), normalize via DVE
      reciprocal+mul, then out[chunk] = sum_h O_h @ Wo_h (bf16, resident
      Wo) accumulated in PSUM and stored straight to OUT.
"""
import sys

try:
    import concourse  # noqa: F401
except ImportError:
    sys.path.insert(0, '/opt/trn_rl_repo')

import ml_dtypes
import numpy as np

import concourse.bacc as bacc
import concourse.mybir as mybir
import concourse.tile as tile
from concourse.bass_utils import run_bass_kernel_spmd

f32 = mybir.dt.float32
f32r = mybir.dt.float32r
bf16 = mybir.dt.bfloat16
AF = mybir.ActivationFunctionType

DIMS = 2048
HEADS = 16
HD = 128           # head dim
WIN = 256          # window / chunk
B, S = 2, 8192
NSH = 4            # seq shards per batch
THETA = 10000.0
ISQ = float(1.0 / np.sqrt(HD))
IB = DIMS // 128   # 16 input-dim blocks


def tok_tiles(n):
    out, a = [], 0
    while a < n:
        w = min(512, n - a)
        out.append((a, w))
        a += w
    return out


def build(nc, T):
    """Emit the per-core program. T = local tokens (multiple of 512)."""
    TH = T + WIN                      # with halo
    NC_ = T // WIN                    # chunks
    NTB = TH // 128                   # 128-token blocks incl halo
    HT = nc.dram_tensor("HT", [DIMS, TH], f32r, kind="ExternalInput")
    WQ = nc.dram_tensor("WQ", [DIMS, DIMS], f32r, kind="ExternalInput")
    WK = nc.dram_tensor("WK", [DIMS, DIMS], f32r, kind="ExternalInput")
    WV = nc.dram_tensor("WV", [DIMS, DIMS], f32r, kind="ExternalInput")
    WO = nc.dram_tensor("WO", [DIMS, DIMS], bf16, kind="ExternalInput")
    COS4 = nc.dram_tensor("COS4", [128, 4, TH], bf16, kind="ExternalInput")
    SIN4 = nc.dram_tensor("SIN4", [128, 4, TH], bf16, kind="ExternalInput")
    TRI23 = nc.dram_tensor("TRI23", [128, 2 * WIN], bf16, kind="ExternalInput")
    PGATE = nc.dram_tensor("PGATE", [128, 1], f32, kind="ExternalInput")
    ONESM = nc.dram_tensor("ONESM", [128, 128], f32r, kind="ExternalInput")
    OUT = nc.dram_tensor("OUT", [T, DIMS], f32, kind="ExternalOutput")

    QTS = nc.dram_tensor("QTS", [HEADS, HD, T], bf16)    # raw (pre-RoPE) Q^T
    KTS = nc.dram_tensor("KTS", [HEADS, HD, TH], bf16)   # raw K^T (with halo)
    VS = nc.dram_tensor("VS", [TH, DIMS], bf16)          # V natural

    with tile.TileContext(nc) as tc:
        with tc.tile_pool(name="cst", bufs=1) as cst:
            tri23 = cst.tile([128, 2 * WIN], bf16)
            pgate = cst.tile([128, 1], f32)
            onesm = cst.tile([128, 128], f32r)
            nc.sync.dma_start(tri23[:], TRI23[:])
            nc.sync.dma_start(pgate[:], PGATE[:])
            nc.sync.dma_start(onesm[:], ONESM[:])

            # ---------------- P1: projections (all f32r) ----------------
            with tc.tile_pool(name="p1", bufs=1) as p1:
                ht = p1.tile([128, IB, TH], f32r)
                htr = HT.rearrange("(ib p) t -> p ib t", p=128)
                NG = 16
                tg = TH // NG
                nc.sync.dma_start(ht[:, :, 0:tg], htr[:, :, 0:tg])

                # V natural: lhsT = hT block [128in, 128tok], rhs = Wv panel
                with tc.tile_pool(name="wv", bufs=2) as wvp, \
                     tc.tile_pool(name="vb", bufs=2) as vbp, \
                     tc.tile_pool(name="vps", bufs=8, space="PSUM") as vps:
                    wvr = WV.rearrange("(ib p) o -> p ib o", p=128)
                    vsr = VS.rearrange("(tb p) c -> p tb c", p=128)
                    for og in range(8):
                        wv = wvp.tile([128, IB, 256], f32r, name="wv")
                        nc.sync.dma_start(
                            wv[:], wvr[:, :, og * 256:(og + 1) * 256])
                        if og == 0:
                            for g in range(1, NG):
                                nc.sync.dma_start(
                                    ht[:, :, g * tg:(g + 1) * tg],
                                    htr[:, :, g * tg:(g + 1) * tg])
                        vbog = vbp.tile([128, NTB, 256], bf16, name="vb")
                        for tb in range(NTB):
                            ps = vps.tile([128, 256], f32, name="vps")
                            for ib in range(IB):
                                nc.tensor.matmul(
                                    ps[:], ht[:, ib, tb * 128:(tb + 1) * 128],
                                    wv[:, ib, :],
                                    start=(ib == 0), stop=(ib == IB - 1))
                            nc.vector.tensor_copy(vbog[:, tb, :], ps[:])
                        nc.scalar.dma_start(
                            vsr[:, :, og * 256:(og + 1) * 256], vbog[:])

                # KT then QT: lhsT = W block [128in, 128out], rhs = hT
                with tc.tile_pool(name="wkq", bufs=3) as wkqp, \
                     tc.tile_pool(name="kst", bufs=3) as kstp, \
                     tc.tile_pool(name="kqps", bufs=8, space="PSUM") as kqps:
                    for W_, DST, t0, tlen in ((WK, KTS, 0, TH),
                                              (WQ, QTS, WIN, T)):
                        wr = W_.rearrange("(ib p) o -> p ib o", p=128)
                        tts = tok_tiles(tlen)
                        for ob in range(HEADS):
                            wt = wkqp.tile([128, IB, 128], f32r, name="wkq")
                            nc.sync.dma_start(
                                wt[:], wr[:, :, ob * 128:(ob + 1) * 128])
                            st = kstp.tile([128, TH], bf16, name="kst")
                            psums = [kqps.tile([128, 512], f32, name="kqps")
                                     for _ in tts]
                            for ib in range(IB):
                                for ti, (a, w) in enumerate(tts):
                                    nc.tensor.matmul(
                                        psums[ti][:, :w], wt[:, ib, :],
                                        ht[:, ib, t0 + a:t0 + a + w],
                                        start=(ib == 0), stop=(ib == IB - 1))
                            for ti, (a, w) in enumerate(tts):
                                nc.scalar.copy(st[:, a:a + w], psums[ti][:, :w])
                            nc.scalar.dma_start(DST[ob][:, :], st[:, :tlen])

            # ---------------- P2 + P3 fused ----------------
            with tc.tile_pool(name="wop", bufs=1) as wop, \
                 tc.tile_pool(name="qk", bufs=2) as qk, \
                 tc.tile_pool(name="rt", bufs=2) as rt, \
                 tc.tile_pool(name="tp", bufs=2) as tp, \
                 tc.tile_pool(name="ptp", bufs=2) as ptp, \
                 tc.tile_pool(name="pa", bufs=2) as pap, \
                 tc.tile_pool(name="ob", bufs=2) as obp, \
                 tc.tile_pool(name="ot", bufs=9) as otp, \
                 tc.tile_pool(name="so3", bufs=2) as so3p, \
                 tc.tile_pool(name="ps_s", bufs=2, space="PSUM") as ps_s, \
                 tc.tile_pool(name="ps_d", bufs=2, space="PSUM") as ps_d, \
                 tc.tile_pool(name="ps_o", bufs=2, space="PSUM") as ps_o, \
                 tc.tile_pool(name="pp3", bufs=2, space="PSUM") as pp3:
                wo = wop.tile([128, IB, DIMS], bf16)
                wor = WO.rearrange("(ib p) o -> p ib o", p=128)

                def rope_loads(specs):
                    """specs: list of (SRC, c0, roped, pos0). Loads each
                    [128, HEADS, WIN] token window from head-major scratch
                    and applies RoPE into `roped` (bf16). DMAs and DVE ops
                    are interleaved at 4-head-group granularity across all
                    specs so the first heads of every tensor unblock after
                    one round."""
                    tiles = []
                    cs_cache = {}
                    for (SRC, c0, roped, pos0) in specs:
                        if pos0 is None:
                            pos0 = c0
                        raw = rt.tile([128, HEADS, WIN], bf16, name="raw")
                        rot = rt.tile([128, HEADS, WIN], bf16, name="rot")
                        if pos0 not in cs_cache:
                            cos4 = tp.tile([128, 4, WIN], bf16, name="cos4")
                            sin4 = tp.tile([128, 4, WIN], bf16, name="sin4")
                            nc.sync.dma_start(cos4[:],
                                              COS4[:, :, pos0:pos0 + WIN])
                            nc.sync.dma_start(sin4[:],
                                              SIN4[:, :, pos0:pos0 + WIN])
                            cs_cache[pos0] = (cos4, sin4)
                        cos4, sin4 = cs_cache[pos0]
                        tiles.append((raw, rot, cos4, sin4))
                    for g in range(4):
                        gs = slice(g * 4, (g + 1) * 4)
                        for (SRC, c0, roped, pos0), (raw, rot, cos4, sin4) \
                                in zip(specs, tiles):
                            sg = SRC[g * 4:(g + 1) * 4, :, c0:c0 + WIN]
                            nc.sync.dma_start(
                                raw[:, gs], sg.rearrange("h d w -> d h w"))
                            nc.sync.dma_start(
                                rot[0:64, gs],
                                sg[:, 64:128, :].rearrange("h d w -> d h w"))
                            nc.sync.dma_start(
                                rot[64:128, gs],
                                sg[:, 0:64, :].rearrange("h d w -> d h w"))
                            tmp = tp.tile([128, 4, WIN], bf16, name="tmp")
                            nc.vector.tensor_mul(tmp[:], rot[:, gs], sin4[:])
                            nc.vector.tensor_mul(roped[:, gs], raw[:, gs],
                                                 cos4[:])
                            nc.vector.tensor_add(roped[:, gs], roped[:, gs],
                                                 tmp[:])

                def v_load(c0, vt):
                    """Cast-load [128, 2, DIMS] V window (bf16 -> f32r) in
                    4-head-group slices so early heads' PV never waits on
                    the whole 4MB transfer."""
                    src = VS[c0:c0 + WIN].rearrange("(tb p) c -> p tb c",
                                                    p=128)
                    for g in range(4):
                        cs = slice(g * 512, (g + 1) * 512)
                        nc.gpsimd.dma_start(vt[:, :, cs], src[:, :, cs])

                kt_prev = qk.tile([128, HEADS, WIN], bf16, name="kt")
                rope_loads([(KTS, 0, kt_prev, None)])
                v_prev = qk.tile([128, 2, DIMS], f32r, name="v")
                v_load(0, v_prev)

                W2 = 2 * WIN
                for c in range(NC_):
                    qt = qk.tile([128, HEADS, WIN], bf16, name="qt")
                    kt_cur = qk.tile([128, HEADS, WIN], bf16, name="kt")
                    rope_loads([(QTS, c * WIN, qt, WIN + c * WIN),
                                (KTS, WIN + c * WIN, kt_cur, None)])
                    v_cur = qk.tile([128, 2, DIMS], f32r, name="v")
                    v_load(WIN + c * WIN, v_cur)
                    if c == 0:
                        # wo quarters, emitted after chunk-0 prep so its bulk
                        # doesn't delay the seam-critical rope loads
                        for g in range(4):
                            nc.sync.dma_start(
                                wo[:, :, g * 512:(g + 1) * 512],
                                wor[:, :, g * 512:(g + 1) * 512])

                    kts = [kt_prev, kt_prev, kt_cur, kt_cur]
                    vs = [v_prev, v_prev, v_cur, v_cur]
                    ots = []
                    for h0 in range(0, HEADS, 2):
                        pd = ps_d.tile([128, W2], f32, name="pd")
                        po = ps_o.tile([128, W2], f32, name="po")
                        pbs2, pads = [], []
                        for h in (h0, h0 + 1):
                            pbs = []
                            for pr in range(2):
                                ps = ps_s.tile([128, W2], f32, name="ps")
                                for kb2 in range(2):
                                    kb = pr * 2 + kb2
                                    nc.tensor.matmul(
                                        ps[:, kb2 * WIN:(kb2 + 1) * WIN],
                                        kts[kb][:, h,
                                                (kb % 2) * 128:(kb % 2) * 128 + 128],
                                        qt[:, h], start=True, stop=True)
                                pb = ptp.tile([128, W2], f32r, name=f"pt{pr}")
                                if pr == 0:
                                    if c == 0:
                                        nc.scalar.activation(
                                            pb[:], ps[:], AF.Exp,
                                            bias=pgate[:], scale=ISQ)
                                    else:
                                        nc.scalar.activation(
                                            pb[:], ps[:], AF.Exp, scale=ISQ)
                                else:
                                    nc.scalar.activation(pb[:], ps[:], AF.Exp,
                                                         scale=ISQ)
                                    nc.vector.tensor_mul(pb[:], pb[:],
                                                         tri23[:])
                                pbs.append(pb)
                            pbs2.append(pbs)
                            pad = pap.tile([128, W2], f32r, name="pad")
                            nc.gpsimd.tensor_add(pad[:], pbs[0][:], pbs[1][:])
                            pads.append(pad)

                        for i, h in enumerate((h0, h0 + 1)):
                            sl = slice(i * WIN, (i + 1) * WIN)
                            for half in range(2):
                                nc.tensor.matmul(
                                    pd[:, sl], onesm[:],
                                    pads[i][:, half * WIN:(half + 1) * WIN],
                                    start=(half == 0), stop=(half == 1))
                            for kb in range(4):
                                pb = pbs2[i][kb // 2][
                                    :, (kb % 2) * WIN:(kb % 2 + 1) * WIN]
                                nc.tensor.matmul(
                                    po[:, sl],
                                    vs[kb][:, kb % 2, h * 128:(h + 1) * 128],
                                    pb, start=(kb == 0), stop=(kb == 3))
                        rb = obp.tile([128, W2], f32, name="rb")
                        with nc.allow_low_precision("softmax denominator"):
                            nc.vector.reciprocal(rb[:], pd[:])
                        ot = otp.tile([128, W2], bf16, name="ot")
                        nc.vector.tensor_mul(ot[:], po[:], rb[:])
                        ots.append(ot)

                    # P3 for this chunk's 256 tokens
                    for tt in range(2):
                        for nt in range(4):
                            ps3 = pp3.tile([128, 512], f32, name="pp3")
                            for h in range(HEADS):
                                hp, i = divmod(h, 2)
                                lhs = ots[hp][:, i * WIN + tt * 128:
                                              i * WIN + tt * 128 + 128]
                                nc.tensor.matmul(
                                    ps3[:], lhs,
                                    wo[:, h, nt * 512:(nt + 1) * 512],
                                    start=(h == 0), stop=(h == HEADS - 1))
                            so = so3p.tile([128, 512], f32, name="so")
                            nc.scalar.copy(so[:], ps3[:])
                            nc.scalar.dma_start(
                                OUT[c * WIN + tt * 128:
                                    c * WIN + (tt + 1) * 128,
                                    nt * 512:(nt + 1) * 512], so[:])
                    kt_prev, v_prev = kt_cur, v_cur
    return nc


def _host_inputs(hidden_states, Wq, Wk, Wv, Wo, T):
    """Build the 8 per-core input maps."""
    TH = T + WIN
    inv_freq = 1.0 / (THETA ** (np.arange(0, HD, 2, dtype=np.float32) / HD))

    qq = np.arange(WIN)[None, :]
    kk = np.arange(128)[:, None]
    tri23 = np.concatenate([(qq >= kk), (qq >= kk + 128)], 1).astype(
        ml_dtypes.bfloat16)
    onesm_f32 = np.ones((128, 128), np.float32)

    wq32, wk32, wv32 = (np.ascontiguousarray(w, np.float32)
                        for w in (Wq, Wk, Wv))
    wo_bf = np.ascontiguousarray(np.asarray(Wo).astype(ml_dtypes.bfloat16))
    in_maps = []
    for core in range(8):
        b, sh = divmod(core, NSH)
        t0 = sh * T
        hs = np.zeros((TH, DIMS), np.float32)
        lo = max(0, t0 - WIN)
        hs[WIN - (t0 - lo):] = hidden_states[b, lo:t0 + T]
        hT = np.ascontiguousarray(hs.T)

        pos = np.arange(t0 - WIN, t0 + T, dtype=np.float32)
        f = np.outer(inv_freq, pos)                      # [64, TH]
        cos = np.concatenate([np.cos(f), np.cos(f)], 0)  # [128, TH]
        sin = np.sin(f)
        sins = np.concatenate([-sin, sin], 0)
        cos4 = np.ascontiguousarray(
            np.broadcast_to(cos[:, None, :], (128, 4, TH)).astype(
                ml_dtypes.bfloat16))
        sin4 = np.ascontiguousarray(
            np.broadcast_to(sins[:, None, :], (128, 4, TH)).astype(
                ml_dtypes.bfloat16))
        pg = np.full((128, 1), -1e30 if sh == 0 else 0.0, np.float32)
        in_maps.append({
            "HT": hT, "WQ": wq32, "WK": wk32, "WV": wv32, "WO": wo_bf,
            "COS4": cos4, "SIN4": sin4,
            "TRI23": tri23, "PGATE": pg, "ONESM": onesm_f32,
        })
    return in_maps


_CACHE = {}


def run(hidden_states, Wq, Wk, Wv, Wo, T=S // NSH, **spmd_kwargs):
    key = T
    if key not in _CACHE:
        nc = bacc.Bacc(None)
        build(nc, T)
        nc.finalize()
        _CACHE[key] = nc
    nc = _CACHE[key]
    in_maps = _host_inputs(hidden_states, Wq, Wk, Wv, Wo, T)
    res = run_bass_kernel_spmd(nc, in_maps, core_ids=list(range(8)),
                               **spmd_kwargs)
    outs = [res.results[i]["OUT"] for i in range(8)]
    full = np.empty((B, NSH * T, DIMS), np.float32)
    for core in range(8):
        b, sh = divmod(core, NSH)
        full[b, sh * T:(sh + 1) * T] = outs[core]
    return full, res


def kernel(hidden_states, Wq, Wk, Wv, Wo):
    out, _ = run(np.asarray(hidden_states), Wq, Wk, Wv, Wo)
    return out
